# revision 7
# baseline (speedup 1.0000x reference)
"""Vocab-parallel projection + cross-entropy loss kernel for TRN2 (8 NeuronCores).

Problem: x [2,2048,2048] f32, y [2,2048] int64, W [128000,2048] f32
  loss = mean_n( logsumexp_v(x_n . W_v) - x_n . W_{y_n} )

Sharding: W's vocab dim split 8 ways (16000 rows/core). Each core computes
  out_s[n] = sum_{v in shard} exp(logit[n, v])     (no max subtraction; logits ~ N(0, 1/3))
  out_t[n] = (y_n in shard) ? logit[n, y_n] : 0
Host combine: loss = mean(log(sum_i out_s_i) - sum_i out_t_i).

Per-core device kernel:
  - cast x, W_shard f32->bf16 into DRAM via SWDGE cast-DMA
  - XBAR transpose-load x^T (SBUF-resident, [128h x 16k x 4096tok] bf16)
  - per vocab tile (512): transpose-load W^T slab, 16 bf16 matmuls per
    128-token block accumulating [128tok x 512v] logits in PSUM,
    then one ScalarE Exp with accum_out -> per-(block,tile) partial sums
  - true logits: indirect-DMA gather of W[y_n] rows (f32) + fused
    multiply-reduce on VectorE, masked by validity
"""

import numpy as np

B, S, H, V = 2, 2048, 2048, 128000
N_CORES = 8
N_TOK = B * S                 # 4096
V_SHARD = V // N_CORES        # 16000
P = 128
V_TILE = 512                  # one PSUM bank of f32

_KERNEL_CACHE = {}


def _build(n_tok, h, vsh, debug=False, do_true=True, do_main=True):
    """Build + compile the single-core SPMD Bass program."""
    import concourse.bass as bass
    import concourse.mybir as mybir
    import concourse.tile as tile
    from concourse import bacc

    kt = h // P                       # k-tiles over hidden dim
    n_tb = n_tok // P                 # token blocks
    # vocab tiles: V_TILE plus remainder (must be multiple of 16 for XBAR)
    v_sizes = [V_TILE] * (vsh // V_TILE)
    if vsh % V_TILE:
        v_sizes.append(vsh % V_TILE)
    n_vt = len(v_sizes)

    nc = bacc.Bacc("TRN2", target_bir_lowering=False, debug=debug)
    f32 = mybir.dt.float32
    bf16 = mybir.dt.bfloat16

    x_in = nc.dram_tensor("x", [n_tok, h], f32, kind="ExternalInput")
    w_in = nc.dram_tensor("w", [vsh, h], f32, kind="ExternalInput")
    sel_in = nc.dram_tensor("sel", [n_tok], mybir.dt.int32, kind="ExternalInput")
    valid_in = nc.dram_tensor("valid", [n_tok], f32, kind="ExternalInput")
    out_s = nc.dram_tensor("out_s", [n_tok], f32, kind="ExternalOutput")
    out_t = nc.dram_tensor("out_t", [n_tok], f32, kind="ExternalOutput")

    xb = nc.dram_tensor("xb", [n_tok, h], bf16)      # bf16 copy of x
    wb = nc.dram_tensor("wb", [vsh, h], bf16)        # bf16 copy of W shard

    with tile.TileContext(nc) as tc:
        with (
            tc.tile_pool(name="const", bufs=1) as cpool,
            tc.tile_pool(name="wslab", bufs=2) as wpool,
            tc.tile_pool(name="psum", bufs=8, space="PSUM") as ppool,
            tc.tile_pool(name="gath", bufs=2) as gpool,
            tc.tile_pool(name="xrow", bufs=2) as xpool,
            tc.tile_pool(name="junk", bufs=1) as jpool,
        ):
            # ---- persistent SBUF tensors ----
            xT = cpool.tile([P, kt, n_tok], bf16, tag="xT")
            sacc = cpool.tile([P, n_tb, n_vt], f32, tag="sacc")
            tacc = cpool.tile([P, n_tb], f32, tag="tacc")
            sel_sb = cpool.tile([P, n_tb], mybir.dt.int32, tag="sel")
            valid_sb = cpool.tile([P, n_tb], f32, tag="valid")
            s2 = cpool.tile([P, n_tb], f32, tag="s2")

            # load per-token metadata: token n = tb*128 + p  ->  [p, tb]
            nc.sync.dma_start(sel_sb[:], sel_in[:].rearrange("(a b) -> b a", b=P))
            nc.sync.dma_start(valid_sb[:], valid_in[:].rearrange("(a b) -> b a", b=P))

            # ---- phase T: true logits (independent of main loop; f32) ----
            for tb in range(n_tb if do_true else 0):
                wg = gpool.tile([P, h], f32, tag="wg")
                nc.gpsimd.indirect_dma_start(
                    out=wg[:],
                    out_offset=None,
                    in_=w_in[:],
                    in_offset=bass.IndirectOffsetOnAxis(ap=sel_sb[:, tb : tb + 1], axis=0),
                )
                xf = xpool.tile([P, h], f32, tag="xf")
                nc.sync.dma_start(xf[:], x_in[tb * P : (tb + 1) * P, :])
                junk = jpool.tile([P, h], bf16, tag="junk")
                nc.vector.tensor_tensor(
                    out=junk[:], in0=xf[:], in1=wg[:], op=mybir.AluOpType.mult
                )
                nc.vector.tensor_reduce(
                    out=tacc[:, tb : tb + 1],
                    in_=junk[:],
                    axis=mybir.AxisListType.X,
                    op=mybir.AluOpType.add,
                )
            if do_true:
                # mask out tokens whose label is not in this shard
                nc.vector.tensor_tensor(
                    out=tacc[:], in0=tacc[:], in1=valid_sb[:], op=mybir.AluOpType.mult
                )
                nc.sync.dma_start(out_t[:].rearrange("(a b) -> b a", b=P), tacc[:])

            # ---- phase 0: bf16 casts (DRAM->DRAM via SWDGE) ----
            if do_main:
                nc.gpsimd.dma_start(xb[:], x_in[:])
            # x^T: 16 XBAR transpose loads [n_tok, 128] -> [128, n_tok]
            for k in range(kt if do_main else 0):
                nc.sync.dma_start_transpose(
                    xT[:, k, :], xb[:, k * P : (k + 1) * P]
                )

            # ---- phase 1: main matmul + exp loop ----
            v0 = 0
            for vt, vsz in enumerate(v_sizes if do_main else []):
                nc.gpsimd.dma_start(wb[v0 : v0 + vsz, :], w_in[v0 : v0 + vsz, :])
                wslab = wpool.tile([P, kt, V_TILE], bf16, tag="wslab")
                for k in range(kt):
                    nc.sync.dma_start_transpose(
                        wslab[:, k, :vsz], wb[v0 : v0 + vsz, k * P : (k + 1) * P]
                    )
                for tb in range(n_tb):
                    psum = ppool.tile([P, V_TILE], f32, tag="psum")
                    for k in range(kt):
                        nc.tensor.matmul(
                            psum[:, :vsz],
                            lhsT=xT[:, k, tb * P : (tb + 1) * P],
                            rhs=wslab[:, k, :vsz],
                            start=(k == 0),
                            stop=(k == kt - 1),
                        )
                    # exp in place (PSUM), free-dim sum -> sacc[:, tb, vt]
                    nc.scalar.activation(
                        out=psum[:, :vsz],
                        in_=psum[:, :vsz],
                        func=mybir.ActivationFunctionType.Exp,
                        accum_out=sacc[:, tb, vt : vt + 1],
                    )
                v0 += vsz

            # ---- phase 2: finalize s ----
            if do_main:
                nc.vector.tensor_reduce(
                    out=s2[:], in_=sacc[:], axis=mybir.AxisListType.X, op=mybir.AluOpType.add
                )
                nc.sync.dma_start(out_s[:].rearrange("(a b) -> b a", b=P), s2[:])

    nc.compile()
    return nc


def _get_kernel(n_tok, h, vsh, debug=False):
    key = (n_tok, h, vsh, debug)
    if key not in _KERNEL_CACHE:
        _KERNEL_CACHE[key] = _build(n_tok, h, vsh, debug=debug)
    return _KERNEL_CACHE[key]


def make_in_maps(x, y, W, n_cores=N_CORES):
    """Shard full inputs into per-core input maps."""
    n_tok = x.shape[0] * x.shape[1] if x.ndim == 3 else x.shape[0]
    h = x.shape[-1]
    v = W.shape[0]
    vsh = v // n_cores
    xf = np.ascontiguousarray(x.reshape(n_tok, h), dtype=np.float32)
    yf = y.reshape(n_tok).astype(np.int64)
    in_maps = []
    for c in range(n_cores):
        lo, hi = c * vsh, (c + 1) * vsh
        owned = (yf >= lo) & (yf < hi)
        sel = np.where(owned, yf - lo, 0).astype(np.int32)
        valid = owned.astype(np.float32)
        in_maps.append(
            {
                "x": xf,
                "w": np.ascontiguousarray(W[lo:hi], dtype=np.float32),
                "sel": sel,
                "valid": valid,
            }
        )
    return in_maps


def combine(results):
    """Host-side unshard: reduce per-core partials to the scalar loss."""
    s = np.sum([r["out_s"].astype(np.float64) for r in results], axis=0)
    t = np.sum([r["out_t"].astype(np.float64) for r in results], axis=0)
    return np.float32(np.mean(np.log(s) - t))


def run_sharded(x, y, W, trace=False):
    from concourse.bass_utils import run_bass_kernel_spmd

    n_tok = x.reshape(-1, x.shape[-1]).shape[0]
    h = x.shape[-1]
    vsh = W.shape[0] // N_CORES
    nc = _get_kernel(n_tok, h, vsh)
    in_maps = make_in_maps(x, y, W)
    res = run_bass_kernel_spmd(nc, in_maps, list(range(N_CORES)), trace=trace)
    return res


def kernel(x, y, W):
    res = run_sharded(np.asarray(x), np.asarray(y), np.asarray(W))
    return combine(res.results)
